# revision 21
# baseline (speedup 1.0000x reference)
"""GAT message-passing kernel for Trainium2 (8 NeuronCores, batch data-parallel).

out[b,i,:] = sum_j softmax_j(mask(leaky_relu(el_i + er_j))) * h[b,j,:] + x[b,i,:]
  h = x @ W, el = x @ (W a_l), er = x @ (W a_r)
  mask: ADJ_BASE*adj_mask + I > 0.1

Layout: rows (b,n) flattened; tiles of 120 rows = 10 graphs; 8 tiles form one
"super-tile" for the attention elementwise chain ([120, 96] ops).

v3: software-pipelined emission — supertile st's attention/aggregation PE ops
are interleaved into supertile st+1's projection-matmul stream (half-tile
granularity) so the tensor engine never idles on the cross-engine attention
chain. Alpha transpose is done as nt cheap matmuls (lhsT=alpha_slice,
rhs=I) straight into a [12, nt*120] PSUM tile. Output stored bf16. Batched
DMAs (1 xn + 4 transposed xt + 1 out store per supertile; adj preloaded).
"""

import numpy as np
import ml_dtypes
from contextlib import ExitStack

import concourse.bass as bass
import concourse.bacc as bacc
import concourse.tile as tile
from concourse import mybir
from concourse.ap import AP
from concourse.bass_utils import run_bass_kernel_spmd
from concourse.bass_test_utils import get_trn_type

N = 12
C = 512
KC = C // 128            # 4 contraction chunks
NEG_SLOPE = 0.2
THRED = 0.1
N_CORES = 8
TILE_R = 120             # rows per matmul tile (10 graphs)
G_PER_TILE = TILE_R // N
ST_TILES = 8             # tiles per super-tile
JW = N * ST_TILES        # 96
XT_COLS = 1024           # per-k-chunk column block in the transposed x tile
BF16 = mybir.dt.bfloat16
F32 = mybir.dt.float32
NPBF16 = ml_dtypes.bfloat16

ADJ_BASE = np.array([
    [0,0,0,1,0,1,1,1,1,1,1,1],
    [0,0,0,1,0,1,1,1,1,1,1,1],
    [0,0,0,1,0,1,1,1,1,1,1,1],
    [1,1,1,0,1,1,1,1,1,1,1,1],
    [0,0,0,1,0,1,1,1,1,1,1,1],
    [1,1,1,1,1,0,1,1,1,0,0,0],
    [1,1,1,1,1,1,0,0,0,1,1,1],
    [1,1,1,1,1,1,0,0,0,1,1,1],
    [1,1,1,1,1,1,0,0,0,1,1,1],
    [1,1,1,1,1,0,1,1,1,0,0,0],
    [1,1,1,1,1,0,1,1,1,0,0,0],
    [1,1,1,1,1,0,1,1,1,0,0,0]], dtype=np.float32)


def host_consts():
    bo = np.kron(np.eye(G_PER_TILE, dtype=np.float32),
                 np.ones((N, N), dtype=np.float32))           # [120,120]
    tidT = np.tile(np.eye(N, dtype=np.float32), (1, G_PER_TILE))  # [12,120]
    adjb = np.tile(ADJ_BASE, (G_PER_TILE, ST_TILES))              # [120,96]
    idm = np.tile(np.eye(N, dtype=np.float32), (G_PER_TILE, ST_TILES))  # [120,96]
    i120 = np.eye(TILE_R, dtype=np.float32)                       # [120,120]
    return {
        "bo": bo.astype(NPBF16),
        "tidT": tidT.astype(NPBF16),
        "adjb": adjb.astype(np.float32),
        "idm": idm.astype(np.float32),
        "i120": i120.astype(NPBF16),
    }


def build_nc(n_tiles: int):
    """Build the per-core Bass program for n_tiles tiles of TILE_R rows."""
    rows = n_tiles * TILE_R
    rows_x = rows + 64       # transposed loads read up to ceil(nt*120/128)*128
    n_st = (n_tiles + ST_TILES - 1) // ST_TILES
    nc = bacc.Bacc(get_trn_type() or "TRN2", target_bir_lowering=False)
    nc.detect_race_conditions = False

    x_d = nc.declare_dram_parameter("x_bf", [rows_x, C], BF16, False)
    am_d = nc.declare_dram_parameter("adj", [n_st * TILE_R, JW], F32, False)
    w_d = nc.declare_dram_parameter("w_bf", [C, C], BF16, False)
    wlr_d = nc.declare_dram_parameter("wlr_bf", [C, 2], BF16, False)
    bo_d = nc.declare_dram_parameter("bo", [TILE_R, TILE_R], BF16, False)
    tidT_d = nc.declare_dram_parameter("tidT", [N, TILE_R], BF16, False)
    adjb_d = nc.declare_dram_parameter("adjb", [TILE_R, JW], F32, False)
    idm_d = nc.declare_dram_parameter("idm", [TILE_R, JW], F32, False)
    i120_d = nc.declare_dram_parameter("i120", [TILE_R, TILE_R], BF16, False)
    out_d = nc.declare_dram_parameter("out", [rows, C], BF16, True)

    with ExitStack() as ctx:
        tc = ctx.enter_context(tile.TileContext(nc))
        _body(ctx, tc, n_tiles, x_d, am_d, w_d, wlr_d,
              bo_d, tidT_d, adjb_d, idm_d, i120_d, out_d)
    nc.compile()
    return nc


def _body(ctx, tc, n_tiles, x_d, am_d, w_d, wlr_d,
          bo_d, tidT_d, adjb_d, idm_d, i120_d, out_d):
    nc = tc.nc
    n_st = (n_tiles + ST_TILES - 1) // ST_TILES

    cpool = ctx.enter_context(tc.tile_pool(name="consts", bufs=1))
    w_sb = cpool.tile([128, KC * C], BF16, name="w_sb")
    wlr_sb = cpool.tile([128, KC * 2], BF16, name="wlr_sb")
    for k in range(KC):
        nc.scalar.dma_start(w_sb[:, k * C:(k + 1) * C], w_d[128 * k:128 * (k + 1), :])
        nc.scalar.dma_start(wlr_sb[:, 2 * k:2 * k + 2], wlr_d[128 * k:128 * (k + 1), :])
    bo_sb = cpool.tile([TILE_R, TILE_R], BF16, name="bo_sb")
    nc.scalar.dma_start(bo_sb[:], bo_d[:])
    tidT_sb = cpool.tile([N, TILE_R], BF16, name="tidT_sb")
    nc.scalar.dma_start(tidT_sb[:], tidT_d[:])
    adjb_sb = cpool.tile([TILE_R, JW], F32, name="adjb_sb")
    nc.scalar.dma_start(adjb_sb[:], adjb_d[:])
    idm_sb = cpool.tile([TILE_R, JW], F32, name="idm_sb")
    nc.scalar.dma_start(idm_sb[:], idm_d[:])
    i120_sb = cpool.tile([TILE_R, TILE_R], BF16, name="i120_sb")
    nc.scalar.dma_start(i120_sb[:], i120_d[:])
    # all adjacency rows, preloaded once: [120, n_st*96] (emitted after the
    # first supertile's x loads so it doesn't delay the pipeline start)
    am_all = cpool.tile([TILE_R, n_st * JW], F32, name="am_all")

    def emit_am_load():
        am_src = am_d[:].rearrange("(S p) j -> S p j", p=TILE_R)
        nc.scalar.dma_start(
            am_all[:].rearrange("p (S j) -> p S j", j=JW),
            am_src.transpose([1, 0, 2]))

    xn_pool = ctx.enter_context(tc.tile_pool(name="xn", bufs=4))
    xt_pool = ctx.enter_context(tc.tile_pool(name="xt", bufs=3))
    h_pool = ctx.enter_context(tc.tile_pool(name="h", bufs=26))
    o_pool = ctx.enter_context(tc.tile_pool(name="o", bufs=2))
    at_pool = ctx.enter_context(tc.tile_pool(name="attn", bufs=3))
    bd_pool = ctx.enter_context(tc.tile_pool(name="bd", bufs=3))
    ph_pool = ctx.enter_context(tc.tile_pool(name="ph", bufs=2, space="PSUM"))
    pe_pool = ctx.enter_context(tc.tile_pool(name="pe", bufs=2, space="PSUM"))
    ps_pool = ctx.enter_context(tc.tile_pool(name="ps", bufs=1, space="PSUM"))
    pa_pool = ctx.enter_context(tc.tile_pool(name="pa", bufs=1, space="PSUM"))
    pg_pool = ctx.enter_context(tc.tile_pool(name="pg", bufs=2, space="PSUM"))

    def st_nt(st):
        return min(ST_TILES, n_tiles - st * ST_TILES)

    def emit_A_dmas(st):
        nt = st_nt(st)
        r0 = st * ST_TILES * TILE_R
        xt_free = ((nt * TILE_R + 127) // 128) * 128
        xn_sup = xn_pool.tile([TILE_R, ST_TILES * C], BF16, tag="xn")
        xn_src = (x_d[0:n_tiles * TILE_R, :]
                  .rearrange("(T p) c -> T p c", p=TILE_R)
                  [st * ST_TILES:st * ST_TILES + nt])
        nc.gpsimd.dma_start(
            xn_sup[:].rearrange("p (T c) -> p T c", c=C)[:, 0:nt],
            xn_src.transpose([1, 0, 2]))
        xt_sup = xt_pool.tile([128, KC * XT_COLS], BF16, tag="xt")
        for k in range(KC):
            nc.sync.dma_start(
                out=xt_sup[:, k * XT_COLS:k * XT_COLS + xt_free],
                in_=x_d[r0:r0 + xt_free, 128 * k:128 * (k + 1)],
                transpose=True)
        elr_ps = pe_pool.tile([TILE_R, 2 * ST_TILES], F32, tag="elr")
        return {"st": st, "nt": nt, "xn": xn_sup, "xt": xt_sup,
                "elr_ps": elr_ps, "h": []}

    def emit_A_half(s, t, half):
        """Half a projection tile: 2 (ph, elr) matmul pairs."""
        xt_sup = s["xt"]
        if half == 0:
            ph = ph_pool.tile([TILE_R, C], F32, tag="ph")
            s["ph"] = ph
        ph = s["ph"]
        for k in (0, 1) if half == 0 else (2, 3):
            lhsT = xt_sup[:, k * XT_COLS + t * TILE_R:
                          k * XT_COLS + (t + 1) * TILE_R]
            nc.tensor.matmul(ph[:], lhsT, w_sb[:, k * C:(k + 1) * C],
                             start=(k == 0), stop=(k == KC - 1))
            nc.tensor.matmul(s["elr_ps"][:, 2 * t:2 * t + 2], lhsT,
                             wlr_sb[:, 2 * k:2 * k + 2],
                             start=(k == 0), stop=(k == KC - 1))
        if half == 1:
            h_sb = h_pool.tile([TILE_R, C], BF16, tag="h")
            nc.scalar.copy(h_sb[:], ph[:])
            s["h"].append(h_sb)

    def make_B_pieces(s):
        """Attention + aggregation for supertile s, as a list of closures."""
        st, nt = s["st"], s["nt"]
        jw = N * nt
        am_sup = am_all[:, st * JW:st * JW + jw]

        def p0_prep():
            elr_sb = at_pool.tile([TILE_R, 2 * ST_TILES], F32, tag="elr_sb")
            s["elr_sb"] = elr_sb
            nc.vector.tensor_copy(elr_sb[:, 0:2 * nt], s["elr_ps"][:, 0:2 * nt])
            rhs_tid = at_pool.tile([TILE_R, JW], BF16, tag="rhs_tid")
            s["rhs_tid"] = rhs_tid
            idm3 = idm_sb[:].rearrange("p (T j) -> p T j", j=N)[:, 0:nt]
            er3 = elr_sb[:, 1:2 * nt:2].unsqueeze(2).broadcast_to([TILE_R, nt, N])
            nc.vector.tensor_tensor(
                rhs_tid[:].rearrange("p (T j) -> p T j", j=N)[:, 0:nt],
                idm3, er3, mybir.AluOpType.mult)
            el8 = at_pool.tile([TILE_R, ST_TILES], F32, tag="el8")
            s["el8"] = el8
            nc.vector.tensor_copy(el8[:, 0:nt], elr_sb[:, 0:2 * nt:2])
            # pass = (adj_mask > 0.1)*ADJ_BASE + I  (no data deps upstream)
            q = at_pool.tile([TILE_R, JW], F32, tag="q")
            nc.vector.scalar_tensor_tensor(
                q[:, 0:jw], am_sup, THRED, adjb_sb[:, 0:jw],
                mybir.AluOpType.is_gt, mybir.AluOpType.mult)
            pass_ = at_pool.tile([TILE_R, JW], F32, tag="pass")
            s["pass"] = pass_
            nc.vector.tensor_tensor(pass_[:, 0:jw], q[:, 0:jw], idm_sb[:, 0:jw],
                                    mybir.AluOpType.add)

        def p1_eb():
            eb_ps = ps_pool.tile([TILE_R, JW], F32, tag="small")
            s["eb_ps"] = eb_ps
            nc.tensor.matmul(eb_ps[:, 0:jw], bo_sb[:], s["rhs_tid"][:, 0:jw],
                             start=True, stop=True)

        def p2_chain():
            e_sb = at_pool.tile([TILE_R, JW], F32, tag="e_sb")
            el3 = s["el8"][:, 0:nt].unsqueeze(2).broadcast_to([TILE_R, nt, N])
            nc.vector.tensor_tensor(
                e_sb[:].rearrange("p (T j) -> p T j", j=N)[:, 0:nt],
                s["eb_ps"][:, 0:jw].rearrange("p (T j) -> p T j", j=N),
                el3, mybir.AluOpType.add)
            e2 = at_pool.tile([TILE_R, JW], F32, tag="e2")
            nc.vector.scalar_tensor_tensor(
                e2[:, 0:jw], e_sb[:, 0:jw], NEG_SLOPE, e_sb[:, 0:jw],
                mybir.AluOpType.mult, mybir.AluOpType.max)
            expv = at_pool.tile([TILE_R, JW], F32, tag="expv")
            nc.scalar.activation(expv[:, 0:jw], e2[:, 0:jw],
                                 mybir.ActivationFunctionType.Exp)
            alphau = at_pool.tile([TILE_R, JW], BF16, tag="alphau")
            s["alphau"] = alphau
            nc.vector.tensor_tensor(alphau[:, 0:jw], expv[:, 0:jw],
                                    s["pass"][:, 0:jw], mybir.AluOpType.mult)
            s8 = at_pool.tile([TILE_R, ST_TILES], F32, tag="s8")
            nc.vector.tensor_reduce(
                s8[:, 0:nt],
                alphau[:].rearrange("p (T j) -> p T j", j=N)[:, 0:nt],
                mybir.AxisListType.X, mybir.AluOpType.add)
            recip8 = at_pool.tile([TILE_R, ST_TILES], F32, tag="recip8")
            s["recip8"] = recip8
            nc.vector.reciprocal(recip8[:, 0:nt], s8[:, 0:nt])

        def p3_aT():
            # alpha^T per tile via matmul: out[j,(g,i)] = alpha[(g,i),(t,j)]
            # split in two psum groups so paT fits one PSUM bank
            aT_sb = at_pool.tile([N, ST_TILES * TILE_R], BF16,
                                 tag="aT_sb", name="aT_sb")
            s["aT_sb"] = aT_sb
            half = (nt + 1) // 2
            for g0 in range(0, nt, half):
                g1 = min(nt, g0 + half)
                paT = pa_pool.tile([N, 4 * TILE_R], F32, tag="paT",
                                   name="paT")
                for t in range(g0, g1):
                    nc.tensor.matmul(
                        paT[:, (t - g0) * TILE_R:(t - g0 + 1) * TILE_R],
                        s["alphau"][:, N * t:N * (t + 1)], i120_sb[:],
                        start=True, stop=True)
                nc.scalar.copy(aT_sb[:, g0 * TILE_R:g1 * TILE_R],
                               paT[:, 0:(g1 - g0) * TILE_R])

        def bdrep_mask(t):
            bdrep = ps_pool.tile([TILE_R, TILE_R], F32, tag="small")
            nc.tensor.matmul(bdrep[:], tidT_sb[:],
                             s["aT_sb"][:, t * TILE_R:(t + 1) * TILE_R],
                             start=True, stop=True)
            bd_sb = bd_pool.tile([TILE_R, TILE_R], BF16, tag="bd")
            nc.vector.tensor_tensor(bd_sb[:], bdrep[:], bo_sb[:],
                                    mybir.AluOpType.mult)
            s.setdefault("bd", {})[t] = bd_sb

        def agg_combine(t):
            pagg = pg_pool.tile([TILE_R, C], F32, tag="pagg")
            nc.tensor.matmul(pagg[:], s["bd"][t][:], s["h"][t][:],
                             start=True, stop=True)
            if t + 1 < nt:
                bdrep_mask(t + 1)
            nc.vector.scalar_tensor_tensor(
                s["out_sup"][:, t * C:(t + 1) * C], pagg[:],
                s["recip8"][:, t:t + 1], s["xn"][:, t * C:(t + 1) * C],
                mybir.AluOpType.mult, mybir.AluOpType.add)

        def p4_first_bd():
            s["out_sup"] = o_pool.tile([TILE_R, ST_TILES * C], BF16,
                                       tag="out_sup", name="out_sup")
            bdrep_mask(0)

        def store():
            out_dst = (out_d[:].rearrange("(T p) c -> T p c", p=TILE_R)
                       [st * ST_TILES:st * ST_TILES + nt])
            nc.gpsimd.dma_start(
                out_dst.transpose([1, 0, 2]),
                s["out_sup"][:].rearrange("p (T c) -> p T c", c=C)[:, 0:nt])

        rest = [p1_eb, p2_chain, p3_aT, p4_first_bd]
        for t in range(nt):
            rest.append(lambda t=t: agg_combine(t))
        rest.append(store)
        return p0_prep, rest

    # ---- software-pipelined emission (B lags A by two supertiles) ----
    # window w: A(w) half-tiles interleaved with rest_B(w-2); p0(w) at the
    # window tail. All B(w) inputs are thus a full supertile period old when
    # the PE consumes them.
    states = {}
    p0s = {}
    rests = {}

    states[0] = emit_A_dmas(0)
    emit_am_load()
    for t in range(states[0]["nt"]):
        emit_A_half(states[0], t, 0)
        emit_A_half(states[0], t, 1)
    p0s[0], rests[0] = make_B_pieces(states[0])
    p0s[0]()

    for w in range(1, n_st):
        sN = emit_A_dmas(w)
        states[w] = sN
        npos = 2 * sN["nt"]
        placed = {}
        tail = []
        if w - 2 >= 0:
            nt_b = states[w - 2]["nt"]
            slots = [1, 2, 4, 6] + list(range(7, 7 + nt_b)) + [15]
            for piece, slot in zip(rests[w - 2], slots):
                if slot < npos:
                    placed.setdefault(slot, []).append(piece)
                else:
                    tail.append(piece)
        pos = 0
        for t in range(sN["nt"]):
            for half in (0, 1):
                emit_A_half(sN, t, half)
                for p in placed.get(pos, ()):
                    p()
                pos += 1
        for p in tail:
            p()
        p0s[w], rests[w] = make_B_pieces(sN)
        p0s[w]()
        if w - 2 >= 0:
            del states[w - 2]

    for w in (n_st - 2, n_st - 1):
        if w >= 0 and w in rests:
            for p in rests[w]:
                p()


_NC_CACHE = {}


def _get_nc(n_tiles):
    if n_tiles not in _NC_CACHE:
        _NC_CACHE[n_tiles] = build_nc(n_tiles)
    return _NC_CACHE[n_tiles]


def prep_core_inputs(x, adj_mask, W, a_l, a_r):
    """Host-side prep: cast, pad, shard. Returns (in_maps, rows_real, n_tiles)."""
    B = x.shape[0]
    assert B % N_CORES == 0
    bpc = B // N_CORES
    rows_real = bpc * N
    n_tiles = (rows_real + TILE_R - 1) // TILE_R
    rows = n_tiles * TILE_R
    rows_x = rows + 64
    n_st = (n_tiles + ST_TILES - 1) // ST_TILES

    Wf = np.asarray(W, dtype=np.float32)
    wl = Wf @ np.asarray(a_l, dtype=np.float32)
    wr = Wf @ np.asarray(a_r, dtype=np.float32)
    w_bf = Wf.astype(NPBF16)
    wlr_bf = np.stack([wl, wr], axis=1).astype(NPBF16)
    consts = host_consts()

    x_bf_full = np.asarray(x, dtype=np.float32).astype(NPBF16)
    adj_full = np.asarray(adj_mask, dtype=np.float32)

    in_maps = []
    for c in range(N_CORES):
        xs = x_bf_full[c * bpc:(c + 1) * bpc].reshape(rows_real, C)
        xp = np.zeros((rows_x, C), dtype=NPBF16)
        xp[:rows_real] = xs
        ams = adj_full[c * bpc:(c + 1) * bpc].reshape(rows_real, N)
        # super-tile layout: amp[st*120 + p, t*12 + j] = adj[(st*8+t)*120 + p, j]
        amp = np.zeros((n_st * ST_TILES * TILE_R, N), dtype=np.float32)
        amp[:rows_real] = ams
        amp = amp.reshape(n_st, ST_TILES, TILE_R, N).transpose(0, 2, 1, 3)
        amp = np.ascontiguousarray(amp).reshape(n_st * TILE_R, ST_TILES * N)
        in_maps.append({
            "x_bf": xp, "adj": amp, "w_bf": w_bf, "wlr_bf": wlr_bf,
            "bo": consts["bo"], "tidT": consts["tidT"], "adjb": consts["adjb"],
            "idm": consts["idm"], "i120": consts["i120"],
        })
    return in_maps, rows_real, n_tiles


def kernel(x, adj_mask, W, a_l, a_r):
    x = np.asarray(x)
    B = x.shape[0]
    in_maps, rows_real, n_tiles = prep_core_inputs(x, adj_mask, W, a_l, a_r)
    nc = _get_nc(n_tiles)
    res = run_bass_kernel_spmd(nc, in_maps, list(range(N_CORES)))
    bpc = B // N_CORES
    outs = [np.asarray(res.results[c]["out"][:rows_real]).reshape(bpc, N, C)
            for c in range(N_CORES)]
    return np.concatenate(outs, axis=0).astype(np.float32, copy=False)


# revision 23
# speedup vs baseline: 1.2567x; 1.2567x over previous
"""GAT message-passing kernel for Trainium2 (8 NeuronCores, batch data-parallel).

out[b,i,:] = sum_j softmax_j(mask(leaky_relu(el_i + er_j))) * h[b,j,:] + x[b,i,:]
  h = x @ W, el = x @ (W a_l), er = x @ (W a_r)
  mask: ADJ_BASE*adj_mask + I > 0.1

Layout: rows (b,n) flattened; tiles of 120 rows = 10 graphs; 8 tiles form one
"super-tile" for the attention elementwise chain ([120, 96] ops).

v3: software-pipelined emission — supertile st's attention/aggregation PE ops
are interleaved into supertile st+1's projection-matmul stream (half-tile
granularity) so the tensor engine never idles on the cross-engine attention
chain. Alpha transpose is done as nt cheap matmuls (lhsT=alpha_slice,
rhs=I) straight into a [12, nt*120] PSUM tile. Output stored bf16. Batched
DMAs (1 xn + 4 transposed xt + 1 out store per supertile; adj preloaded).
"""

import numpy as np
import ml_dtypes
from contextlib import ExitStack

import concourse.bass as bass
import concourse.bacc as bacc
import concourse.tile as tile
from concourse import mybir
from concourse.ap import AP
from concourse.bass_utils import run_bass_kernel_spmd
from concourse.bass_test_utils import get_trn_type

N = 12
C = 512
KC = C // 128            # 4 contraction chunks
NEG_SLOPE = 0.2
THRED = 0.1
N_CORES = 8
TILE_R = 120             # rows per matmul tile (10 graphs)
G_PER_TILE = TILE_R // N
ST_TILES = 8             # tiles per super-tile
JW = N * ST_TILES        # 96
XT_COLS = 1024           # per-k-chunk column block in the transposed x tile
BF16 = mybir.dt.bfloat16
F32 = mybir.dt.float32
NPBF16 = ml_dtypes.bfloat16

ADJ_BASE = np.array([
    [0,0,0,1,0,1,1,1,1,1,1,1],
    [0,0,0,1,0,1,1,1,1,1,1,1],
    [0,0,0,1,0,1,1,1,1,1,1,1],
    [1,1,1,0,1,1,1,1,1,1,1,1],
    [0,0,0,1,0,1,1,1,1,1,1,1],
    [1,1,1,1,1,0,1,1,1,0,0,0],
    [1,1,1,1,1,1,0,0,0,1,1,1],
    [1,1,1,1,1,1,0,0,0,1,1,1],
    [1,1,1,1,1,1,0,0,0,1,1,1],
    [1,1,1,1,1,0,1,1,1,0,0,0],
    [1,1,1,1,1,0,1,1,1,0,0,0],
    [1,1,1,1,1,0,1,1,1,0,0,0]], dtype=np.float32)


def host_consts():
    bo = np.kron(np.eye(G_PER_TILE, dtype=np.float32),
                 np.ones((N, N), dtype=np.float32))           # [120,120]
    tidT = np.tile(np.eye(N, dtype=np.float32), (1, G_PER_TILE))  # [12,120]
    adjb = np.tile(ADJ_BASE, (G_PER_TILE, ST_TILES))              # [120,96]
    idm = np.tile(np.eye(N, dtype=np.float32), (G_PER_TILE, ST_TILES))  # [120,96]
    i120 = np.eye(TILE_R, dtype=np.float32)                       # [120,120]
    return {
        "bo": bo.astype(NPBF16),
        "tidT": tidT.astype(NPBF16),
        "adjb": adjb.astype(np.float32),
        "idm": idm.astype(np.float32),
        "i120": i120.astype(NPBF16),
    }


def build_nc(n_tiles: int):
    """Build the per-core Bass program for n_tiles tiles of TILE_R rows."""
    rows = n_tiles * TILE_R
    rows_x = rows + 64       # transposed loads read up to ceil(nt*120/128)*128
    n_st = (n_tiles + ST_TILES - 1) // ST_TILES
    nc = bacc.Bacc(get_trn_type() or "TRN2", target_bir_lowering=False)
    nc.detect_race_conditions = False

    x_d = nc.declare_dram_parameter("x_bf", [rows_x, C], BF16, False)
    am_d = nc.declare_dram_parameter("adj", [n_st * TILE_R, JW], F32, False)
    w_d = nc.declare_dram_parameter("w_bf", [C, C], BF16, False)
    wlr_d = nc.declare_dram_parameter("wlr_bf", [C, 2], BF16, False)
    bo_d = nc.declare_dram_parameter("bo", [TILE_R, TILE_R], BF16, False)
    tidT_d = nc.declare_dram_parameter("tidT", [N, TILE_R], BF16, False)
    adjb_d = nc.declare_dram_parameter("adjb", [TILE_R, JW], F32, False)
    idm_d = nc.declare_dram_parameter("idm", [TILE_R, JW], F32, False)
    i120_d = nc.declare_dram_parameter("i120", [TILE_R, TILE_R], BF16, False)
    out_d = nc.declare_dram_parameter("out", [rows, C], BF16, True)

    with ExitStack() as ctx:
        tc = ctx.enter_context(tile.TileContext(nc))
        _body(ctx, tc, n_tiles, x_d, am_d, w_d, wlr_d,
              bo_d, tidT_d, adjb_d, idm_d, i120_d, out_d)
    nc.compile()
    return nc


def _body(ctx, tc, n_tiles, x_d, am_d, w_d, wlr_d,
          bo_d, tidT_d, adjb_d, idm_d, i120_d, out_d):
    nc = tc.nc
    n_st = (n_tiles + ST_TILES - 1) // ST_TILES

    cpool = ctx.enter_context(tc.tile_pool(name="consts", bufs=1))
    w_sb = cpool.tile([128, KC * C], BF16, name="w_sb")
    wlr_sb = cpool.tile([128, KC * 2], BF16, name="wlr_sb")
    for k in range(KC):
        nc.scalar.dma_start(w_sb[:, k * C:(k + 1) * C], w_d[128 * k:128 * (k + 1), :])
        nc.scalar.dma_start(wlr_sb[:, 2 * k:2 * k + 2], wlr_d[128 * k:128 * (k + 1), :])
    bo_sb = cpool.tile([TILE_R, TILE_R], BF16, name="bo_sb")
    nc.scalar.dma_start(bo_sb[:], bo_d[:])
    tidT_sb = cpool.tile([N, TILE_R], BF16, name="tidT_sb")
    nc.scalar.dma_start(tidT_sb[:], tidT_d[:])
    adjb_sb = cpool.tile([TILE_R, JW], F32, name="adjb_sb")
    nc.scalar.dma_start(adjb_sb[:], adjb_d[:])
    idm_sb = cpool.tile([TILE_R, JW], F32, name="idm_sb")
    nc.scalar.dma_start(idm_sb[:], idm_d[:])
    i120_sb = cpool.tile([TILE_R, TILE_R], BF16, name="i120_sb")
    nc.scalar.dma_start(i120_sb[:], i120_d[:])
    # all adjacency rows, preloaded once: [120, n_st*96] (emitted after the
    # first supertile's x loads so it doesn't delay the pipeline start)
    am_all = cpool.tile([TILE_R, n_st * JW], F32, name="am_all")

    def emit_am_load():
        am_src = am_d[:].rearrange("(S p) j -> S p j", p=TILE_R)
        nc.scalar.dma_start(
            am_all[:].rearrange("p (S j) -> p S j", j=JW),
            am_src.transpose([1, 0, 2]))

    xn_pool = ctx.enter_context(tc.tile_pool(name="xn", bufs=4))
    xt_pool = ctx.enter_context(tc.tile_pool(name="xt", bufs=3))
    h_pool = ctx.enter_context(tc.tile_pool(name="h", bufs=26))
    o_pool = ctx.enter_context(tc.tile_pool(name="o", bufs=2))
    at_pool = ctx.enter_context(tc.tile_pool(name="attn", bufs=3))
    bd_pool = ctx.enter_context(tc.tile_pool(name="bd", bufs=3))
    ph_pool = ctx.enter_context(tc.tile_pool(name="ph", bufs=2, space="PSUM"))
    pe_pool = ctx.enter_context(tc.tile_pool(name="pe", bufs=2, space="PSUM"))
    ps_pool = ctx.enter_context(tc.tile_pool(name="ps", bufs=2, space="PSUM"))
    pg_pool = ctx.enter_context(tc.tile_pool(name="pg", bufs=2, space="PSUM"))

    def st_nt(st):
        return min(ST_TILES, n_tiles - st * ST_TILES)

    def emit_A_dmas(st):
        nt = st_nt(st)
        r0 = st * ST_TILES * TILE_R
        xt_free = ((nt * TILE_R + 127) // 128) * 128
        xn_sup = xn_pool.tile([TILE_R, ST_TILES * C], BF16, tag="xn")
        xn_src = (x_d[0:n_tiles * TILE_R, :]
                  .rearrange("(T p) c -> T p c", p=TILE_R)
                  [st * ST_TILES:st * ST_TILES + nt])
        nc.sync.dma_start(
            xn_sup[:].rearrange("p (T c) -> p T c", c=C)[:, 0:nt],
            xn_src.transpose([1, 0, 2]))
        xt_sup = xt_pool.tile([128, KC * XT_COLS], BF16, tag="xt")
        for k in range(KC):
            nc.sync.dma_start(
                out=xt_sup[:, k * XT_COLS:k * XT_COLS + xt_free],
                in_=x_d[r0:r0 + xt_free, 128 * k:128 * (k + 1)],
                transpose=True)
        elr_ps = pe_pool.tile([TILE_R, 2 * ST_TILES], F32, tag="elr")
        return {"st": st, "nt": nt, "xn": xn_sup, "xt": xt_sup,
                "elr_ps": elr_ps, "h": []}

    def emit_A_half(s, t, half):
        """Half a projection tile: 2 (ph, elr) matmul pairs."""
        xt_sup = s["xt"]
        if half == 0:
            ph = ph_pool.tile([TILE_R, C], F32, tag="ph")
            s["ph"] = ph
        ph = s["ph"]
        for k in (0, 1) if half == 0 else (2, 3):
            lhsT = xt_sup[:, k * XT_COLS + t * TILE_R:
                          k * XT_COLS + (t + 1) * TILE_R]
            nc.tensor.matmul(ph[:], lhsT, w_sb[:, k * C:(k + 1) * C],
                             start=(k == 0), stop=(k == KC - 1))
            nc.tensor.matmul(s["elr_ps"][:, 2 * t:2 * t + 2], lhsT,
                             wlr_sb[:, 2 * k:2 * k + 2],
                             start=(k == 0), stop=(k == KC - 1))
        if half == 1:
            h_sb = h_pool.tile([TILE_R, C], BF16, tag="h")
            nc.scalar.copy(h_sb[:], ph[:])
            s["h"].append(h_sb)

    def make_B_pieces(s):
        """Attention + aggregation for supertile s, as a list of closures."""
        st, nt = s["st"], s["nt"]
        jw = N * nt
        am_sup = am_all[:, st * JW:st * JW + jw]

        def p0_prep():
            elr_sb = at_pool.tile([TILE_R, 2 * ST_TILES], F32, tag="elr_sb")
            s["elr_sb"] = elr_sb
            nc.vector.tensor_copy(elr_sb[:, 0:2 * nt], s["elr_ps"][:, 0:2 * nt])
            rhs_tid = at_pool.tile([TILE_R, JW], BF16, tag="rhs_tid")
            s["rhs_tid"] = rhs_tid
            idm3 = idm_sb[:].rearrange("p (T j) -> p T j", j=N)[:, 0:nt]
            er3 = elr_sb[:, 1:2 * nt:2].unsqueeze(2).broadcast_to([TILE_R, nt, N])
            nc.vector.tensor_tensor(
                rhs_tid[:].rearrange("p (T j) -> p T j", j=N)[:, 0:nt],
                idm3, er3, mybir.AluOpType.mult)
            el8 = at_pool.tile([TILE_R, ST_TILES], F32, tag="el8")
            s["el8"] = el8
            nc.vector.tensor_copy(el8[:, 0:nt], elr_sb[:, 0:2 * nt:2])
            # pass = (adj_mask > 0.1)*ADJ_BASE + I  (no data deps upstream)
            q = at_pool.tile([TILE_R, JW], F32, tag="q")
            nc.vector.scalar_tensor_tensor(
                q[:, 0:jw], am_sup, THRED, adjb_sb[:, 0:jw],
                mybir.AluOpType.is_gt, mybir.AluOpType.mult)
            pass_ = at_pool.tile([TILE_R, JW], F32, tag="pass")
            s["pass"] = pass_
            nc.vector.tensor_tensor(pass_[:, 0:jw], q[:, 0:jw], idm_sb[:, 0:jw],
                                    mybir.AluOpType.add)

        def p1_eb():
            eb_ps = ps_pool.tile([TILE_R, JW], F32, tag="small")
            s["eb_ps"] = eb_ps
            nc.tensor.matmul(eb_ps[:, 0:jw], bo_sb[:], s["rhs_tid"][:, 0:jw],
                             start=True, stop=True)

        def p2_chain():
            e_sb = at_pool.tile([TILE_R, JW], F32, tag="e_sb")
            el3 = s["el8"][:, 0:nt].unsqueeze(2).broadcast_to([TILE_R, nt, N])
            nc.vector.tensor_tensor(
                e_sb[:].rearrange("p (T j) -> p T j", j=N)[:, 0:nt],
                s["eb_ps"][:, 0:jw].rearrange("p (T j) -> p T j", j=N),
                el3, mybir.AluOpType.add)
            e2 = at_pool.tile([TILE_R, JW], F32, tag="e2")
            nc.vector.scalar_tensor_tensor(
                e2[:, 0:jw], e_sb[:, 0:jw], NEG_SLOPE, e_sb[:, 0:jw],
                mybir.AluOpType.mult, mybir.AluOpType.max)
            expv = at_pool.tile([TILE_R, JW], F32, tag="expv")
            nc.scalar.activation(expv[:, 0:jw], e2[:, 0:jw],
                                 mybir.ActivationFunctionType.Exp)
            alphau = at_pool.tile([TILE_R, JW], BF16, tag="alphau")
            s["alphau"] = alphau
            nc.vector.tensor_tensor(alphau[:, 0:jw], expv[:, 0:jw],
                                    s["pass"][:, 0:jw], mybir.AluOpType.mult)
            s8 = at_pool.tile([TILE_R, ST_TILES], F32, tag="s8")
            nc.vector.tensor_reduce(
                s8[:, 0:nt],
                alphau[:].rearrange("p (T j) -> p T j", j=N)[:, 0:nt],
                mybir.AxisListType.X, mybir.AluOpType.add)
            recip8 = at_pool.tile([TILE_R, ST_TILES], F32, tag="recip8")
            s["recip8"] = recip8
            nc.vector.reciprocal(recip8[:, 0:nt], s8[:, 0:nt])

        def p3_aT(group):
            # alpha^T per tile via matmul: out[j,(g,i)] = alpha[(g,i),(t,j)]
            # two psum groups so each paT chunk fits one PSUM bank
            if group == 0:
                s["aT_sb"] = at_pool.tile([N, ST_TILES * TILE_R], BF16,
                                          tag="aT_sb", name="aT_sb")
            half = (nt + 1) // 2
            g0 = group * half
            g1 = min(nt, g0 + half)
            if g0 >= g1:
                return
            paT = ps_pool.tile([N, 4 * TILE_R], F32, tag="small", name="paT")
            for t in range(g0, g1):
                nc.tensor.matmul(
                    paT[:, (t - g0) * TILE_R:(t - g0 + 1) * TILE_R],
                    s["alphau"][:, N * t:N * (t + 1)], i120_sb[:],
                    start=True, stop=True)
            nc.scalar.copy(s["aT_sb"][:, g0 * TILE_R:g1 * TILE_R],
                           paT[:, 0:(g1 - g0) * TILE_R])

        def bdrep_mask(t):
            bdrep = ps_pool.tile([TILE_R, TILE_R], F32, tag="small")
            nc.tensor.matmul(bdrep[:], tidT_sb[:],
                             s["aT_sb"][:, t * TILE_R:(t + 1) * TILE_R],
                             start=True, stop=True)
            bd_sb = bd_pool.tile([TILE_R, TILE_R], BF16, tag="bd")
            nc.vector.tensor_tensor(bd_sb[:], bdrep[:], bo_sb[:],
                                    mybir.AluOpType.mult)
            s.setdefault("bd", {})[t] = bd_sb

        def agg_combine(t):
            pagg = pg_pool.tile([TILE_R, C], F32, tag="pagg")
            nc.tensor.matmul(pagg[:], s["bd"][t][:], s["h"][t][:],
                             start=True, stop=True)
            if t + 1 < nt:
                bdrep_mask(t + 1)
            nc.vector.scalar_tensor_tensor(
                s["out_sup"][:, t * C:(t + 1) * C], pagg[:],
                s["recip8"][:, t:t + 1], s["xn"][:, t * C:(t + 1) * C],
                mybir.AluOpType.mult, mybir.AluOpType.add)

        def p4_first_bd():
            s["out_sup"] = o_pool.tile([TILE_R, ST_TILES * C], BF16,
                                       tag="out_sup", name="out_sup")
            bdrep_mask(0)

        def store():
            out_dst = (out_d[:].rearrange("(T p) c -> T p c", p=TILE_R)
                       [st * ST_TILES:st * ST_TILES + nt])
            nc.sync.dma_start(
                out_dst.transpose([1, 0, 2]),
                s["out_sup"][:].rearrange("p (T c) -> p T c", c=C)[:, 0:nt])

        rest = [lambda: p3_aT(0), lambda: p3_aT(1), p4_first_bd]
        for t in range(nt):
            rest.append(lambda t=t: agg_combine(t))
        rest.append(store)
        return p0_prep, p1_eb, p2_chain, rest

    # ---- software-pipelined emission (B lags A by two supertiles) ----
    # Window w emits: DMAs for A(w+1) (so transfers land a full window before
    # use), A(w) half-tiles with rest_B(w-2) pieces spread through them, and
    # a tail of [p0(w), eb(w-1), chain(w-1)] so every B stage's inputs are at
    # least a full window old when its consumer runs.
    states = {}
    pieces = {}   # st -> (p0, p1_eb, p2_chain, rest)

    states[0] = emit_A_dmas(0)
    states[1] = emit_A_dmas(1) if n_st > 1 else None
    emit_am_load()

    for w in range(n_st):
        sW = states[w]
        if w + 2 < n_st:
            states[w + 2] = emit_A_dmas(w + 2)
        npos = 2 * sW["nt"]
        placed = {}
        tail = []
        if w - 2 >= 0:
            nt_b = states[w - 2]["nt"]
            slots = [3, 4, 6] + list(range(7, 7 + nt_b)) + [15]
            for piece, slot in zip(pieces[w - 2][3], slots):
                if slot < npos:
                    placed.setdefault(slot, []).append(piece)
                else:
                    tail.append(piece)
        pos = 0
        for t in range(sW["nt"]):
            for half in (0, 1):
                emit_A_half(sW, t, half)
                for p in placed.get(pos, ()):
                    p()
                pos += 1
        for p in tail:
            p()
        pieces[w] = make_B_pieces(sW)
        pieces[w][0]()                       # p0(w)
        if w - 1 >= 0:
            pieces[w - 1][1]()               # eb(w-1)
            pieces[w - 1][2]()               # chain(w-1)
        if w - 2 >= 0:
            del states[w - 2]

    for w in (n_st - 2, n_st - 1):
        if w < 0 or w not in pieces:
            continue
        if w == n_st - 1:
            pieces[w][1]()                   # eb of the last supertile
            pieces[w][2]()
        for p in pieces[w][3]:
            p()


_NC_CACHE = {}


def _get_nc(n_tiles):
    if n_tiles not in _NC_CACHE:
        _NC_CACHE[n_tiles] = build_nc(n_tiles)
    return _NC_CACHE[n_tiles]


def prep_core_inputs(x, adj_mask, W, a_l, a_r):
    """Host-side prep: cast, pad, shard. Returns (in_maps, rows_real, n_tiles)."""
    B = x.shape[0]
    assert B % N_CORES == 0
    bpc = B // N_CORES
    rows_real = bpc * N
    n_tiles = (rows_real + TILE_R - 1) // TILE_R
    rows = n_tiles * TILE_R
    rows_x = rows + 64
    n_st = (n_tiles + ST_TILES - 1) // ST_TILES

    Wf = np.asarray(W, dtype=np.float32)
    wl = Wf @ np.asarray(a_l, dtype=np.float32)
    wr = Wf @ np.asarray(a_r, dtype=np.float32)
    w_bf = Wf.astype(NPBF16)
    wlr_bf = np.stack([wl, wr], axis=1).astype(NPBF16)
    consts = host_consts()

    x_bf_full = np.asarray(x, dtype=np.float32).astype(NPBF16)
    adj_full = np.asarray(adj_mask, dtype=np.float32)

    in_maps = []
    for c in range(N_CORES):
        xs = x_bf_full[c * bpc:(c + 1) * bpc].reshape(rows_real, C)
        xp = np.zeros((rows_x, C), dtype=NPBF16)
        xp[:rows_real] = xs
        ams = adj_full[c * bpc:(c + 1) * bpc].reshape(rows_real, N)
        # super-tile layout: amp[st*120 + p, t*12 + j] = adj[(st*8+t)*120 + p, j]
        amp = np.zeros((n_st * ST_TILES * TILE_R, N), dtype=np.float32)
        amp[:rows_real] = ams
        amp = amp.reshape(n_st, ST_TILES, TILE_R, N).transpose(0, 2, 1, 3)
        amp = np.ascontiguousarray(amp).reshape(n_st * TILE_R, ST_TILES * N)
        in_maps.append({
            "x_bf": xp, "adj": amp, "w_bf": w_bf, "wlr_bf": wlr_bf,
            "bo": consts["bo"], "tidT": consts["tidT"], "adjb": consts["adjb"],
            "idm": consts["idm"], "i120": consts["i120"],
        })
    return in_maps, rows_real, n_tiles


def kernel(x, adj_mask, W, a_l, a_r):
    x = np.asarray(x)
    B = x.shape[0]
    in_maps, rows_real, n_tiles = prep_core_inputs(x, adj_mask, W, a_l, a_r)
    nc = _get_nc(n_tiles)
    res = run_bass_kernel_spmd(nc, in_maps, list(range(N_CORES)))
    bpc = B // N_CORES
    outs = [np.asarray(res.results[c]["out"][:rows_real]).reshape(bpc, N, C)
            for c in range(N_CORES)]
    return np.concatenate(outs, axis=0).astype(np.float32, copy=False)
